# revision 37
# baseline (speedup 1.0000x reference)
"""Trainium2 Bass kernel for the DGL-JTNN tree-GRU encoder.

Only the bottom-up pass matters for the output (root readout); the down
phase is dead code.  Structure per core (8 trees, data-parallel over trees):

- Host precomputes input-independent vocab tables (pure weight transforms):
    ML[v]  = sigmoid(e_v @ Wz1 + bz) * tanh(e_v @ Wh1 + bh)   (leaf message)
  so the whole leaf z/h~ work collapses into one table gather.
- All graph gathers use dma_gather(transpose=True), which delivers gathered
  table rows feature-major straight into SBUF (no PE transposes, no PSUM
  staging, no copy-out).
- Levels are stored in "parity order": level l+1 columns are [left children
  of level-l order | right children].  Every pair reduction (s = m_l + m_r,
  arm feeds) is then a contiguous half-add: packed APs, DVE 2x mode.
- Everything on-chip is bf16 except PSUM accumulation and biases (f32).
"""

import os
import sys

import numpy as np

for _p in ("/opt/trn_rl_repo",):
    if os.path.isdir(_p) and _p not in sys.path:
        sys.path.insert(0, _p)

B, DEPTH, H, VOCAB = 64, 10, 128, 780
NPT = 2 ** (DEPTH + 1) - 1
NCORES = 8
T = B // NCORES  # trees per core

LN = {l: T * (1 << l) for l in range(DEPTH + 1)}  # cols per level per core
CH = 1024  # chunk width (mm moving / ACT / TT)

# small levels packed into one gather stream: level -> offset in XS tile
_SMALL_LEVELS = [5, 4, 3, 2, 1, 0]
XS_OFF = {}
_o = 0
for _l in _SMALL_LEVELS:
    XS_OFF[_l] = _o
    _o += LN[_l]
XS_COLS = 512  # 504 used + 8 pad
assert _o <= XS_COLS

# Gather issue order (512-idx units).  X9 chunks interleave ahead of the leaf
# chunks that need them (leaf-r reads Wr@X9), so the first leaf-r psum can
# start ~3 gathers in.  Leaf chunk order is pair order [0,4,1,5,2,6,3,7].
# Each entry: (block, start, count) with count a multiple of 512.
LEAF_ORDER = [0, 4, 2, 6, 1, 5, 3, 7]  # leaf CH-chunks (CH=1024)
# level-9 chunk emitted after leaf index i (its ML/RM10 halves are done)
ZH9_AT = {1: 0, 3: 2, 5: 1, 7: 3}


def _gather_schedule():
    sched = []
    x9_emitted = set()
    for i, c in enumerate(LEAF_ORDER):
        x9c = c % 4
        if i == 0:
            # finest interleave up front: first leaf-r half unblocks after
            # two 512-gathers instead of four
            x9_emitted.add(x9c)
            sched += [("x9", 0, 512), ("leaf", 0, 512),
                      ("x9", 512, 512), ("leaf", 512, 512)]
            continue
        if x9c not in x9_emitted:
            x9_emitted.add(x9c)
            sched.append(("x9", x9c * 1024, 1024))
        sched.append(("leaf", c * 1024, 1024))
    sched.append(("x8", 0, 1024))
    sched.append(("x8", 1024, 1024))
    sched.append(("x7", 0, 1024))
    sched.append(("x6", 0, 512))
    sched.append(("xs", 0, 512))
    return sched

GSCHED = _gather_schedule()

# gidx stream layout = gather units in issue order (prefix loads work)
_BLK_LEN = {"leaf": LN[10], "x9": LN[9], "x8": LN[8], "x7": LN[7],
            "x6": LN[6], "xs": XS_COLS}
GTOT = sum(cnt for _, _, cnt in GSCHED)  # 16384 idxs
GCOLS = GTOT // 16  # 1024 int16 cols

_NC_CACHE = {}

# engine knobs: pool | dve.  Pool is FIFO with the gathers, so putting work
# there too early stalls the pipeline behind the gather queue.
RM_BIG_ENG = os.environ.get("DGLJ_RM_BIG", "pool")
RM_SMALL_ENG = os.environ.get("DGLJ_RM_SMALL", "dve")
S_ENG = os.environ.get("DGLJ_S_ENG", "dve")
LEAF_RM_ENG = os.environ.get("DGLJ_LEAF_RM_ENG", "dve")
# levels <= this use the latency-optimized t1-form chain
SMALL_MAX = int(os.environ.get("DGLJ_SMALL_MAX", "6"))


def _pair_order(nch):
    """Process chunks so consumers (which need chunk c and c + nch/2 of this
    level) unblock after two producer chunks."""
    if nch <= 1:
        return list(range(nch))
    h = nch // 2
    out = []
    for i in range(h):
        out += [i, h + i]
    return out


def _parity_patterns():
    """pat[l] = heap ids of level-l nodes in parity order.  Column k of level
    l is (pattern p = k // T, tree t = k % T); left child of col k lives at
    col k of level l+1, right child at col k + LN[l]."""
    pat = {0: [0]}
    for l in range(DEPTH):
        pl = pat[l]
        pat[l + 1] = [2 * h + 1 for h in pl] + [2 * h + 2 for h in pl]
    return pat


_PAT = _parity_patterns()


def _build_nc(reps=1):
    from contextlib import ExitStack

    import concourse.bass as bass
    import concourse.mybir as mybir
    import concourse.tile as tile
    from concourse import bacc

    f32 = mybir.dt.float32
    bf16 = mybir.dt.bfloat16
    i16 = mybir.dt.int16
    AF = mybir.ActivationFunctionType

    nc = bacc.Bacc("TRN2", target_bir_lowering=False)

    emb16_d = nc.dram_tensor("emb16", [VOCAB, H], bf16, kind="ExternalInput")
    mltab_d = nc.dram_tensor("mltab", [VOCAB, H], bf16, kind="ExternalInput")
    gidx_d = nc.dram_tensor("gidx", [128, GCOLS], i16, kind="ExternalInput")
    wpack16_d = nc.dram_tensor("wpack16", [H, 9 * H], bf16, kind="ExternalInput")
    wbias_d = nc.dram_tensor("wbias", [H, 4], f32, kind="ExternalInput")
    out_d = nc.dram_tensor("out", [H, T], f32, kind="ExternalOutput")

    _W = ("wr", "ur", "wz1", "wz2", "wh1", "wh2", "wg1", "wg2", "nur")
    _B = ("bz", "bh", "br", "bg")

    with tile.TileContext(nc) as tc, ExitStack() as ctx:
        consts = ctx.enter_context(tc.tile_pool(name="consts", bufs=1))
        xpool = ctx.enter_context(tc.tile_pool(name="xp", bufs=1))
        mpool = ctx.enter_context(tc.tile_pool(name="mp", bufs=1))
        ck = ctx.enter_context(tc.tile_pool(name="ck", bufs=3))
        pzp = ctx.enter_context(tc.tile_pool(name="pz", bufs=2, space="PSUM"))
        php = ctx.enter_context(tc.tile_pool(name="ph", bufs=2, space="PSUM"))

        # ---- constants (parallel loads: ACT=wbias+warm, SP=gidx/w16) ----
        gidx = consts.tile([128, GCOLS], i16, tag="gidx", name="gidx")
        wbias = consts.tile([H, 4], f32, tag="wb", name="wbias")
        nc.scalar.dma_start(out=wbias[:], in_=wbias_d[:])
        w16 = consts.tile([H, 9 * H], bf16, tag="w16", name="w16")
        # gidx slice loads sized so the earliest gathers unblock fastest;
        # leaf weights (wr, ur = first 2H cols of wpack) squeeze in early
        _gslices = [128, 128, 256, 256, 256]
        _g0 = 0
        for _i, _gs in enumerate(_gslices):
            nc.sync.dma_start(
                out=gidx[:, _g0 : _g0 + _gs], in_=gidx_d[:, _g0 : _g0 + _gs]
            )
            _g0 += _gs
            if _i == 0:
                nc.sync.dma_start(out=w16[:, : 2 * H], in_=wpack16_d[:, : 2 * H])
            elif _i == 1:
                nc.sync.dma_start(out=w16[:, 2 * H :], in_=wpack16_d[:, 2 * H :])
        assert _g0 == GCOLS
        wsb = {n: w16[:, i * H : (i + 1) * H] for i, n in enumerate(_W)}
        bsb = {n: wbias[:, i : i + 1] for i, n in enumerate(_B)}
        # dummy 1-col sigmoid: hoist ACT table load into startup
        warm = consts.tile([H, 1], f32, tag="warm", name="warm")
        nc.scalar.activation(warm[:], wbias[:, :1], AF.Sigmoid)

        MM = 512  # max moving cols per matmul writing PSUM (one bank)

        def accum(psum, w, terms):
            """Accumulate sum of (weight_ap, rhs_fn) into psum[:, :w].

            rhs_fn(s0, sw) returns the moving operand for psum cols
            [s0, s0+sw).  Emitted in 512-col segments (PSUM bank limit),
            each segment its own accumulation group."""
            for s0 in range(0, w, MM):
                sw = min(MM, w - s0)
                for ti, (wap, rhs_fn) in enumerate(terms):
                    nc.tensor.matmul(
                        psum[:, s0 : s0 + sw], wap, rhs_fn(s0, sw),
                        start=(ti == 0), stop=(ti == len(terms) - 1),
                    )

        def gather(dst3, src_d, goff, i0, cnt):
            """Gather cnt rows of src_d (feature-major into dst3[:, :, i0:i0+cnt])
            using gidx stream cols at goff+i0 (goff = stream offset of this
            gather unit).  Transpose-mode gathers are limited to 512 indices
            per instruction (1024 crashes the device)."""
            for s0 in range(0, cnt, 512):
                sc = min(512, cnt - s0)
                c0 = (goff + s0) // 16
                nc.gpsimd.dma_gather(
                    dst3[:, :, i0 + s0 : i0 + s0 + sc],
                    src_d[:, :],
                    gidx[:, c0 : c0 + sc // 16],
                    num_idxs=sc,
                    num_idxs_reg=sc,
                    elem_size=H,
                    transpose=True,
                )

        # ---- X / table tiles (3D: [128, 1, n] so gather APs line up) ----
        ML3 = xpool.tile([128, 1, LN[10]], bf16, tag="ml", name="ML")
        X9_3 = xpool.tile([128, 1, LN[9]], bf16, tag="x9", name="X9")
        X8_3 = xpool.tile([128, 1, LN[8]], bf16, tag="x8", name="X8")
        X7_3 = xpool.tile([128, 1, LN[7]], bf16, tag="x7", name="X7")
        X6_3 = xpool.tile([128, 1, LN[6]], bf16, tag="x6", name="X6")
        XS_3 = xpool.tile([128, 1, XS_COLS], bf16, tag="xs", name="XS")
        ML = ML3[:, 0, :]

        def xview(l):
            if l == 9:
                return X9_3[:, 0, :]
            if l == 8:
                return X8_3[:, 0, :]
            if l == 7:
                return X7_3[:, 0, :]
            if l == 6:
                return X6_3[:, 0, :]
            return XS_3[:, 0, XS_OFF[l] : XS_OFF[l] + LN[l]]

        _dst = {"leaf": (ML3, mltab_d), "x9": (X9_3, emb16_d),
                "x8": (X8_3, emb16_d), "x7": (X7_3, emb16_d),
                "x6": (X6_3, emb16_d), "xs": (XS_3, emb16_d)}

        for _rep in range(reps):
            # ---- gathers (Pool, FIFO): issue in GSCHED order ----
            goff = 0
            for blk, b0, cnt in GSCHED:
                dst3, src_d = _dst[blk]
                gather(dst3, src_d, goff, b0, cnt)
                goff += cnt
            # ---- leaf r/rm:  rm = sigmoid(Wr@x_p + Ur@ML + br) * ML ----
            RM10 = mpool.tile([128, LN[10]], bf16, tag="rm0", name="RM10")
            X9 = xview(9)

            def leaf_chunk(c, i, split=False):
                c0 = c * CH
                x0 = c0 % LN[9]
                eng = nc.gpsimd if LEAF_RM_ENG == "pool" else nc.vector
                # first chunk runs as two 512 halves so the first sigmoid
                # fires one gather-unit earlier
                for k, (off, w) in enumerate(
                    [(0, 512), (512, 512)] if split else [(0, CH)]
                ):
                    pr = (pzp if (i + k) % 2 == 0 else php).tile(
                        [128, CH], f32,
                        tag="pz" if (i + k) % 2 == 0 else "ph", name=f"lpr{c}_{k}"
                    )
                    accum(pr, w, [
                        (wsb["wr"], lambda s0, sw, o=off: X9[:, x0 + o + s0 : x0 + o + s0 + sw]),
                        (wsb["ur"], lambda s0, sw, o=off: ML[:, c0 + o + s0 : c0 + o + s0 + sw]),
                    ])
                    r = ck.tile([128, CH], bf16, tag="r", name=f"lr{c}_{k}")
                    nc.scalar.activation(r[:, :w], pr[:, :w], AF.Sigmoid,
                                         bias=bsb["br"])
                    eng.tensor_mul(RM10[:, c0 + off : c0 + off + w], r[:, :w],
                                   ML[:, c0 + off : c0 + off + w])

            def zh_part(l, c, lch, S, M, Mn, RMn, X, zfirst=False):
                """Bulk chunk.  h~ first by default (its RMn inputs are ready
                at level start); z first when the child level finished just
                before (rm lands after m, so tanh's input is the late one)."""
                n = LN[l]
                c0 = c * lch
                w = min(lch, n - c0)
                seng = nc.gpsimd if S_ENG == "pool" else nc.vector

                def emit_h():
                    ph = php.tile([128, CH], f32, tag="ph", name=f"ph{l}_{c}")
                    accum(ph, w, [
                        (wsb["wh1"], lambda s0, sw: X[:, c0 + s0 : c0 + s0 + sw]),
                        (wsb["wh2"], lambda s0, sw: RMn[:, c0 + s0 : c0 + s0 + sw]),
                        (wsb["wh2"], lambda s0, sw: RMn[:, n + c0 + s0 : n + c0 + s0 + sw]),
                    ])
                    ht = ck.tile([128, CH], bf16, tag="h", name=f"ht{l}_{c}")
                    nc.scalar.activation(ht[:, :w], ph[:, :w], AF.Tanh, bias=bsb["bh"])
                    return ht

                def emit_z():
                    pz = pzp.tile([128, CH], f32, tag="pz", name=f"pz{l}_{c}")
                    accum(pz, w, [
                        (wsb["wz1"], lambda s0, sw: X[:, c0 + s0 : c0 + s0 + sw]),
                        (wsb["wz2"], lambda s0, sw: S[:, c0 + s0 : c0 + s0 + sw]),
                    ])
                    z = ck.tile([128, CH], bf16, tag="z", name=f"z{l}_{c}")
                    nc.scalar.activation(z[:, :w], pz[:, :w], AF.Sigmoid,
                                         bias=bsb["bz"])
                    return z

                if zfirst:
                    z = emit_z()
                    ht = emit_h()
                else:
                    ht = emit_h()
                    z = emit_z()
                # m = s + z*(h~ - s)
                u = ck.tile([128, CH], bf16, tag="u", name=f"u{l}_{c}")
                nc.vector.tensor_sub(u[:, :w], ht[:, :w], S[:, c0 : c0 + w])
                v = ck.tile([128, CH], bf16, tag="v", name=f"v{l}_{c}")
                nc.vector.tensor_mul(v[:, :w], z[:, :w], u[:, :w])
                nc.vector.tensor_add(M[:, c0 : c0 + w], S[:, c0 : c0 + w], v[:, :w])

            def r_part(l, c, lch, S, M, RM, Xp):
                """r = sigmoid(Wr@x_parent + Ur@m + br); rm = r*m."""
                n = LN[l]
                hp = LN[l - 1]
                c0 = c * lch
                w = min(lch, n - c0)
                pr = (php if c % 2 == 0 else pzp).tile(
                    [128, CH], f32, tag="ph" if c % 2 == 0 else "pz",
                    name=f"pr{l}_{c}"
                )
                def xp_rhs(s0, sw):
                    p0 = (c0 + s0) % hp
                    return Xp[:, p0 : p0 + sw]
                accum(pr, w, [
                    (wsb["wr"], xp_rhs),
                    (wsb["ur"], lambda s0, sw: M[:, c0 + s0 : c0 + s0 + sw]),
                ])
                r = ck.tile([128, CH], bf16, tag="r", name=f"r{l}_{c}")
                nc.scalar.activation(r[:, :w], pr[:, :w], AF.Sigmoid, bias=bsb["br"])
                rmeng = nc.gpsimd if RM_BIG_ENG == "pool" else nc.vector
                rmeng.tensor_mul(RM[:, c0 : c0 + w], r[:, :w], M[:, c0 : c0 + w])

            def small_chunk(l, S, M, RM, Mn, RMn, X, Xp):
                """Single-chunk latency form, z first: the m of the child
                level (s, z inputs) lands two hops before its rm (tanh
                input), so σ_z rides in the rm->tanh latency.  σ_r chain:
                tanh → t1 → Ur@t1 → σ_r → rm."""
                n = w = LN[l]
                hp = LN[l - 1]
                # tiny widths: Pool is idle here and has no access bubble
                te = nc.gpsimd if w <= 128 else nc.vector
                seng = nc.gpsimd if (S_ENG == "pool" or w <= 128) else nc.vector
                seng.tensor_add(S[:, :w], Mn[:, :w], Mn[:, n : n + w])
                pz = pzp.tile([128, CH], f32, tag="pz", name=f"pz{l}")
                accum(pz, w, [
                    (wsb["wz1"], lambda s0, sw: X[:, s0 : s0 + sw]),
                    (wsb["wz2"], lambda s0, sw: S[:, s0 : s0 + sw]),
                ])
                z = ck.tile([128, CH], bf16, tag="z", name=f"z{l}")
                nc.scalar.activation(z[:, :w], pz[:, :w], AF.Sigmoid, bias=bsb["bz"])
                ph = php.tile([128, CH], f32, tag="ph", name=f"ph{l}")
                accum(ph, w, [
                    (wsb["wh1"], lambda s0, sw: X[:, s0 : s0 + sw]),
                    (wsb["wh2"], lambda s0, sw: RMn[:, s0 : s0 + sw]),
                    (wsb["wh2"], lambda s0, sw: RMn[:, n + s0 : n + s0 + sw]),
                ])
                ht = ck.tile([128, CH], bf16, tag="h", name=f"ht{l}")
                nc.scalar.activation(ht[:, :w], ph[:, :w], AF.Tanh, bias=bsb["bh"])
                pr = None
                if l >= 2:
                    pr = (php if l % 2 == 0 else pzp).tile(
                        [128, CH], f32, tag="ph" if l % 2 == 0 else "pz",
                        name=f"pr{l}"
                    )
                    if w <= hp:
                        nc.tensor.matmul(pr[:, :w], wsb["wr"], Xp[:, :w],
                                         start=True, stop=False)
                    else:
                        nc.tensor.matmul(pr[:, :hp], wsb["wr"], Xp[:, :hp],
                                         start=True, stop=False)
                        nc.tensor.matmul(pr[:, hp:w], wsb["wr"], Xp[:, :hp],
                                         start=True, stop=False)
                    nc.tensor.matmul(pr[:, :w], wsb["ur"], S[:, :w],
                                     start=False, stop=False)
                t2 = ck.tile([128, CH], bf16, tag="u", name=f"t2{l}")
                te.tensor_mul(t2[:, :w], z[:, :w], S[:, :w])
                mp = ck.tile([128, CH], bf16, tag="v", name=f"mp{l}")
                te.tensor_sub(mp[:, :w], S[:, :w], t2[:, :w])
                if pr is not None:
                    nc.tensor.matmul(pr[:, :w], wsb["nur"], t2[:, :w],
                                     start=False, stop=False)
                t1 = ck.tile([128, CH], bf16, tag="t1", name=f"t1{l}")
                te.tensor_mul(t1[:, :w], z[:, :w], ht[:, :w])
                te.tensor_add(M[:, :w], mp[:, :w], t1[:, :w])
                if pr is not None:
                    nc.tensor.matmul(pr[:, :w], wsb["ur"], t1[:, :w],
                                     start=False, stop=True)
                    r = ck.tile([128, CH], bf16, tag="r", name=f"r{l}")
                    nc.scalar.activation(r[:, :w], pr[:, :w], AF.Sigmoid,
                                         bias=bsb["br"])
                    rmeng = nc.gpsimd if (RM_SMALL_ENG == "pool" or w <= 128) else nc.vector
                    rmeng.tensor_mul(RM[:, :w], r[:, :w], M[:, :w])

            # level-9 tiles: z/h parts run interleaved with the leaf chunks
            # (leaf gathers bound Pool early; keep ACT fed with level-9 work)
            M9 = mpool.tile([128, LN[9]], bf16, tag="m1", name="M9")
            RM9 = mpool.tile([128, LN[9]], bf16, tag="rm1", name="RM9")
            S9 = mpool.tile([128, LN[9]], bf16, tag="s1", name="S9")
            seng9 = nc.gpsimd if S_ENG == "pool" else nc.vector
            # r parts deferred past the X8 gathers; chunks {0,2} right after
            # zh9c1 so RM9{0,2} (level-8's first gate) lands early in the
            # dense ACT window
            R9_AT = {5: (0, 2), 7: (1, 3)}
            for i, c in enumerate(LEAF_ORDER):
                leaf_chunk(c, i, split=(i == 0))
                if i in ZH9_AT:
                    c9 = ZH9_AT[i] * CH
                    seng9.tensor_add(
                        S9[:, c9 : c9 + CH], ML[:, c9 : c9 + CH],
                        ML[:, LN[9] + c9 : LN[9] + c9 + CH]
                    )
                    zh_part(9, ZH9_AT[i], CH, S9, M9, ML, RM10, X9)
                if i in R9_AT:
                    for c9 in R9_AT[i]:
                        r_part(9, c9, CH, S9, M9, RM9, xview(8))

            # ---- levels 8..1 ----
            Mn, RMn = M9, RM9
            M1 = None
            for l in range(8, 0, -1):
                n = LN[l]
                X = xview(l)
                Xp = xview(l - 1)
                M = mpool.tile([128, n], bf16, tag=f"m{l % 2}", name=f"M{l}")
                RM = None
                if l >= 2:
                    RM = mpool.tile([128, n], bf16, tag=f"rm{l % 2}", name=f"RM{l}")
                S = mpool.tile([128, n], bf16, tag=f"s{l % 2}", name=f"S{l}")
                lch = min(512, n)
                nch = (n + lch - 1) // lch
                if l <= SMALL_MAX and nch == 1:
                    small_chunk(l, S, M, RM, Mn, RMn, X, Xp)
                else:
                    seng = nc.gpsimd if S_ENG == "pool" else nc.vector
                    for c in range(nch):
                        c0 = c * lch
                        w = min(lch, n - c0)
                        # s = m_left + m_right; inputs land early, so emit
                        # all chunks up front to keep the DVE queue unblocked
                        seng.tensor_add(
                            S[:, c0 : c0 + w], Mn[:, c0 : c0 + w],
                            Mn[:, n + c0 : n + c0 + w]
                        )
                    porder = _pair_order(nch)
                    for g0 in range(0, nch, 2):
                        grp = porder[g0 : g0 + 2]
                        for c in grp:
                            zh_part(l, c, lch, S, M, Mn, RMn, X,
                                    zfirst=(l <= 7))
                        if l >= 2:
                            for c in grp:
                                r_part(l, c, lch, S, M, RM, Xp)
                Mn, RMn = M, RM
                if l == 1:
                    M1 = M

            # ---- root readout: relu(Wg1@x_root + Wg2@(m_l + m_r) + bg) ----
            # relu+bias folded into one DVE tensor_scalar: max(pg + bg, 0)
            S0 = mpool.tile([128, T], bf16, tag="s0", name="S0")
            nc.gpsimd.tensor_add(S0[:], M1[:, :T], M1[:, T : 2 * T])
            pg = pzp.tile([128, CH], f32, tag="pz", name="pg")
            nc.tensor.matmul(pg[:, :T], wsb["wg1"], xview(0), start=True, stop=False)
            nc.tensor.matmul(pg[:, :T], wsb["wg2"], S0[:], start=False, stop=True)
            outt = ck.tile([128, T], f32, tag="o", name="outt")
            nc.vector.tensor_scalar(
                outt[:], pg[:, :T], bsb["bg"], 0.0,
                op0=mybir.AluOpType.add, op1=mybir.AluOpType.max,
            )
            nc.sync.dma_start(out=out_d[:, :], in_=outt[:])

    nc.finalize()
    return nc


def get_nc(reps=1):
    key = ("nc", reps)
    if key not in _NC_CACHE:
        _NC_CACHE[key] = _build_nc(reps)
    return _NC_CACHE[key]


def _wrap_idx(ids):
    """int16 index stream -> [16, n/16] wrapped layout."""
    return ids.astype(np.int16).reshape(-1, 16).T


def make_core_inputs(wid, emb, weights):
    import ml_dtypes

    bf16 = ml_dtypes.bfloat16
    e = np.asarray(emb, dtype=np.float32)
    Wz, Wh = np.asarray(weights["Wz_w"], np.float32), np.asarray(weights["Wh_w"], np.float32)
    Wr, Ur = np.asarray(weights["Wr_w"], np.float32), np.asarray(weights["Ur_w"], np.float32)
    Wg = np.asarray(weights["Wg_w"], np.float32)
    bz, bh = np.asarray(weights["Wz_b"], np.float32), np.asarray(weights["Wh_b"], np.float32)
    br, bg = np.asarray(weights["Ur_b"], np.float32), np.asarray(weights["Wg_b"], np.float32)

    # vocab tables (input-independent weight transforms)
    zl = 1.0 / (1.0 + np.exp(-(e @ Wz[:H] + bz)))
    hl = np.tanh(e @ Wh[:H] + bh)
    ML32 = (zl * hl).astype(np.float32)
    mltab = np.ascontiguousarray(ML32.astype(bf16))
    emb16 = np.ascontiguousarray(e.astype(bf16))

    wmats = [Wr, Ur, Wz[:H], Wz[H:], Wh[:H], Wh[H:], Wg[:H], Wg[H:], -Ur]
    wpack16 = np.concatenate(wmats, axis=1).astype(bf16)
    wbias = np.stack([bz, bh, br, bg], axis=1).astype(np.float32)

    base = {
        "emb16": emb16,
        "mltab": mltab,
        "wpack16": np.ascontiguousarray(wpack16),
        "wbias": np.ascontiguousarray(wbias),
    }

    wid = np.asarray(wid).reshape(B, NPT)
    in_maps = []
    for cid in range(NCORES):
        widc = wid[cid * T : (cid + 1) * T]  # [T, NPT]
        # per-level wids in parity (pattern-major) order
        def lvl_ids(l):
            pat = np.asarray(_PAT[l])
            return widc[np.arange(T)[None, :], pat[:, None]].ravel()  # [P*T]

        ids_by_blk = {"leaf": lvl_ids(10), "x9": lvl_ids(9), "x8": lvl_ids(8),
                      "x7": lvl_ids(7), "x6": lvl_ids(6)}
        xs = np.concatenate([lvl_ids(l) for l in _SMALL_LEVELS])
        ids_by_blk["xs"] = np.concatenate(
            [xs, np.zeros(XS_COLS - len(xs), np.int64)]
        )
        # stream = gather units in issue order
        ids = np.concatenate(
            [ids_by_blk[blk][b0 : b0 + cnt] for blk, b0, cnt in GSCHED]
        )
        assert len(ids) == GTOT
        gi = _wrap_idx(ids)  # [16, GCOLS]
        in_maps.append({**base, "gidx": np.ascontiguousarray(np.tile(gi, (8, 1)))})
    return in_maps


def kernel(**inputs):
    from concourse.bass_utils import run_bass_kernel_spmd

    nc = get_nc()
    in_maps = make_core_inputs(inputs["wid"], inputs["emb"], inputs)
    res = run_bass_kernel_spmd(nc, in_maps, core_ids=list(range(NCORES)))
    out = np.concatenate(
        [np.asarray(res.results[c]["out"]).T for c in range(NCORES)], axis=0
    )
    return np.ascontiguousarray(out.astype(np.float32))



# revision 38
# speedup vs baseline: 1.0053x; 1.0053x over previous
"""Trainium2 Bass kernel for the DGL-JTNN tree-GRU encoder.

Only the bottom-up pass matters for the output (root readout); the down
phase is dead code.  Structure per core (8 trees, data-parallel over trees):

- Host precomputes input-independent vocab tables (pure weight transforms):
    ML[v]  = sigmoid(e_v @ Wz1 + bz) * tanh(e_v @ Wh1 + bh)   (leaf message)
  so the whole leaf z/h~ work collapses into one table gather.
- All graph gathers use dma_gather(transpose=True), which delivers gathered
  table rows feature-major straight into SBUF (no PE transposes, no PSUM
  staging, no copy-out).
- Levels are stored in "parity order": level l+1 columns are [left children
  of level-l order | right children].  Every pair reduction (s = m_l + m_r,
  arm feeds) is then a contiguous half-add: packed APs, DVE 2x mode.
- Everything on-chip is bf16 except PSUM accumulation and biases (f32).
"""

import os
import sys

import numpy as np

for _p in ("/opt/trn_rl_repo",):
    if os.path.isdir(_p) and _p not in sys.path:
        sys.path.insert(0, _p)

B, DEPTH, H, VOCAB = 64, 10, 128, 780
NPT = 2 ** (DEPTH + 1) - 1
NCORES = 8
T = B // NCORES  # trees per core

LN = {l: T * (1 << l) for l in range(DEPTH + 1)}  # cols per level per core
CH = 1024  # chunk width (mm moving / ACT / TT)

# small levels packed into one gather stream: level -> offset in XS tile
_SMALL_LEVELS = [5, 4, 3, 2, 1, 0]
XS_OFF = {}
_o = 0
for _l in _SMALL_LEVELS:
    XS_OFF[_l] = _o
    _o += LN[_l]
XS_COLS = 512  # 504 used + 8 pad
assert _o <= XS_COLS

# Gather issue order (512-idx units).  X9 chunks interleave ahead of the leaf
# chunks that need them (leaf-r reads Wr@X9), so the first leaf-r psum can
# start ~3 gathers in.  Leaf chunk order is pair order [0,4,1,5,2,6,3,7].
# Each entry: (block, start, count) with count a multiple of 512.
LEAF_ORDER = [0, 4, 2, 6, 1, 5, 3, 7]  # leaf CH-chunks (CH=1024)
# level-9 chunk emitted after leaf index i (its ML/RM10 halves are done)
ZH9_AT = {1: 0, 3: 2, 5: 1, 7: 3}


def _gather_schedule():
    sched = []
    x9_emitted = set()
    for i, c in enumerate(LEAF_ORDER):
        x9c = c % 4
        if i == 0:
            # finest interleave up front: first leaf-r half unblocks after
            # two 512-gathers instead of four
            x9_emitted.add(x9c)
            sched += [("x9", 0, 512), ("leaf", 0, 512),
                      ("x9", 512, 512), ("leaf", 512, 512)]
            continue
        if x9c not in x9_emitted:
            x9_emitted.add(x9c)
            sched.append(("x9", x9c * 1024, 1024))
        sched.append(("leaf", c * 1024, 1024))
    sched.append(("x8", 0, 1024))
    sched.append(("x8", 1024, 1024))
    sched.append(("x7", 0, 1024))
    sched.append(("x6", 0, 512))
    sched.append(("xs", 0, 512))
    return sched

GSCHED = _gather_schedule()

# gidx stream layout = gather units in issue order (prefix loads work)
_BLK_LEN = {"leaf": LN[10], "x9": LN[9], "x8": LN[8], "x7": LN[7],
            "x6": LN[6], "xs": XS_COLS}
GTOT = sum(cnt for _, _, cnt in GSCHED)  # 16384 idxs
GCOLS = GTOT // 16  # 1024 int16 cols

_NC_CACHE = {}

# engine knobs: pool | dve.  Pool is FIFO with the gathers, so putting work
# there too early stalls the pipeline behind the gather queue.
RM_BIG_ENG = os.environ.get("DGLJ_RM_BIG", "pool")
RM_SMALL_ENG = os.environ.get("DGLJ_RM_SMALL", "dve")
S_ENG = os.environ.get("DGLJ_S_ENG", "dve")
LEAF_RM_ENG = os.environ.get("DGLJ_LEAF_RM_ENG", "dve")
# levels <= this use the latency-optimized t1-form chain
SMALL_MAX = int(os.environ.get("DGLJ_SMALL_MAX", "6"))


def _pair_order(nch):
    """Process chunks so consumers (which need chunk c and c + nch/2 of this
    level) unblock after two producer chunks."""
    if nch <= 1:
        return list(range(nch))
    h = nch // 2
    out = []
    for i in range(h):
        out += [i, h + i]
    return out


def _parity_patterns():
    """pat[l] = heap ids of level-l nodes in parity order.  Column k of level
    l is (pattern p = k // T, tree t = k % T); left child of col k lives at
    col k of level l+1, right child at col k + LN[l]."""
    pat = {0: [0]}
    for l in range(DEPTH):
        pl = pat[l]
        pat[l + 1] = [2 * h + 1 for h in pl] + [2 * h + 2 for h in pl]
    return pat


_PAT = _parity_patterns()


def _build_nc(reps=1):
    from contextlib import ExitStack

    import concourse.bass as bass
    import concourse.mybir as mybir
    import concourse.tile as tile
    from concourse import bacc

    f32 = mybir.dt.float32
    bf16 = mybir.dt.bfloat16
    i16 = mybir.dt.int16
    AF = mybir.ActivationFunctionType

    nc = bacc.Bacc("TRN2", target_bir_lowering=False)

    emb16_d = nc.dram_tensor("emb16", [VOCAB, H], bf16, kind="ExternalInput")
    mltab_d = nc.dram_tensor("mltab", [VOCAB, H], bf16, kind="ExternalInput")
    gidx_d = nc.dram_tensor("gidx", [128, GCOLS], i16, kind="ExternalInput")
    wpack16_d = nc.dram_tensor("wpack16", [H, 9 * H], bf16, kind="ExternalInput")
    wbias_d = nc.dram_tensor("wbias", [H, 4], f32, kind="ExternalInput")
    out_d = nc.dram_tensor("out", [H, T], f32, kind="ExternalOutput")

    _W = ("wr", "ur", "wz1", "wz2", "wh1", "wh2", "wg1", "wg2", "nur")
    _B = ("bz", "bh", "br", "bg")

    with tile.TileContext(nc) as tc, ExitStack() as ctx:
        consts = ctx.enter_context(tc.tile_pool(name="consts", bufs=1))
        xpool = ctx.enter_context(tc.tile_pool(name="xp", bufs=1))
        mpool = ctx.enter_context(tc.tile_pool(name="mp", bufs=1))
        ck = ctx.enter_context(tc.tile_pool(name="ck", bufs=3))
        pzp = ctx.enter_context(tc.tile_pool(name="pz", bufs=2, space="PSUM"))
        php = ctx.enter_context(tc.tile_pool(name="ph", bufs=2, space="PSUM"))

        # ---- constants (parallel loads: ACT=wbias+warm, SP=gidx/w16) ----
        gidx = consts.tile([128, GCOLS], i16, tag="gidx", name="gidx")
        wbias = consts.tile([H, 4], f32, tag="wb", name="wbias")
        nc.scalar.dma_start(out=wbias[:], in_=wbias_d[:])
        w16 = consts.tile([H, 9 * H], bf16, tag="w16", name="w16")
        # gidx slice loads sized so the earliest gathers unblock fastest;
        # leaf weights (wr, ur = first 2H cols of wpack) squeeze in early
        _gslices = [128, 128, 256, 256, 256]
        _g0 = 0
        for _i, _gs in enumerate(_gslices):
            nc.sync.dma_start(
                out=gidx[:, _g0 : _g0 + _gs], in_=gidx_d[:, _g0 : _g0 + _gs]
            )
            _g0 += _gs
            if _i == 0:
                nc.sync.dma_start(out=w16[:, : 2 * H], in_=wpack16_d[:, : 2 * H])
            elif _i == 1:
                nc.sync.dma_start(out=w16[:, 2 * H :], in_=wpack16_d[:, 2 * H :])
        assert _g0 == GCOLS
        wsb = {n: w16[:, i * H : (i + 1) * H] for i, n in enumerate(_W)}
        bsb = {n: wbias[:, i : i + 1] for i, n in enumerate(_B)}
        # dummy 1-col sigmoid: hoist ACT table load into startup
        warm = consts.tile([H, 1], f32, tag="warm", name="warm")
        nc.scalar.activation(warm[:], wbias[:, :1], AF.Sigmoid)

        MM = 512  # max moving cols per matmul writing PSUM (one bank)

        def accum(psum, w, terms):
            """Accumulate sum of (weight_ap, rhs_fn) into psum[:, :w].

            rhs_fn(s0, sw) returns the moving operand for psum cols
            [s0, s0+sw).  Emitted in 512-col segments (PSUM bank limit),
            each segment its own accumulation group."""
            for s0 in range(0, w, MM):
                sw = min(MM, w - s0)
                for ti, (wap, rhs_fn) in enumerate(terms):
                    nc.tensor.matmul(
                        psum[:, s0 : s0 + sw], wap, rhs_fn(s0, sw),
                        start=(ti == 0), stop=(ti == len(terms) - 1),
                    )

        def gather(dst3, src_d, goff, i0, cnt):
            """Gather cnt rows of src_d (feature-major into dst3[:, :, i0:i0+cnt])
            using gidx stream cols at goff+i0 (goff = stream offset of this
            gather unit).  Transpose-mode gathers are limited to 512 indices
            per instruction (1024 crashes the device)."""
            for s0 in range(0, cnt, 512):
                sc = min(512, cnt - s0)
                c0 = (goff + s0) // 16
                nc.gpsimd.dma_gather(
                    dst3[:, :, i0 + s0 : i0 + s0 + sc],
                    src_d[:, :],
                    gidx[:, c0 : c0 + sc // 16],
                    num_idxs=sc,
                    num_idxs_reg=sc,
                    elem_size=H,
                    transpose=True,
                )

        # ---- X / table tiles (3D: [128, 1, n] so gather APs line up) ----
        ML3 = xpool.tile([128, 1, LN[10]], bf16, tag="ml", name="ML")
        X9_3 = xpool.tile([128, 1, LN[9]], bf16, tag="x9", name="X9")
        X8_3 = xpool.tile([128, 1, LN[8]], bf16, tag="x8", name="X8")
        X7_3 = xpool.tile([128, 1, LN[7]], bf16, tag="x7", name="X7")
        X6_3 = xpool.tile([128, 1, LN[6]], bf16, tag="x6", name="X6")
        XS_3 = xpool.tile([128, 1, XS_COLS], bf16, tag="xs", name="XS")
        ML = ML3[:, 0, :]

        def xview(l):
            if l == 9:
                return X9_3[:, 0, :]
            if l == 8:
                return X8_3[:, 0, :]
            if l == 7:
                return X7_3[:, 0, :]
            if l == 6:
                return X6_3[:, 0, :]
            return XS_3[:, 0, XS_OFF[l] : XS_OFF[l] + LN[l]]

        _dst = {"leaf": (ML3, mltab_d), "x9": (X9_3, emb16_d),
                "x8": (X8_3, emb16_d), "x7": (X7_3, emb16_d),
                "x6": (X6_3, emb16_d), "xs": (XS_3, emb16_d)}

        for _rep in range(reps):
            # ---- gathers (Pool, FIFO): issue in GSCHED order ----
            goff = 0
            for blk, b0, cnt in GSCHED:
                dst3, src_d = _dst[blk]
                gather(dst3, src_d, goff, b0, cnt)
                goff += cnt
            # ---- leaf r/rm:  rm = sigmoid(Wr@x_p + Ur@ML + br) * ML ----
            RM10 = mpool.tile([128, LN[10]], bf16, tag="rm0", name="RM10")
            X9 = xview(9)

            def leaf_chunk(c, i, split=False):
                c0 = c * CH
                x0 = c0 % LN[9]
                eng = nc.gpsimd if LEAF_RM_ENG == "pool" else nc.vector
                # first chunk runs as two 512 halves so the first sigmoid
                # fires one gather-unit earlier
                for k, (off, w) in enumerate(
                    [(0, 512), (512, 512)] if split else [(0, CH)]
                ):
                    pr = (pzp if (i + k) % 2 == 0 else php).tile(
                        [128, CH], f32,
                        tag="pz" if (i + k) % 2 == 0 else "ph", name=f"lpr{c}_{k}"
                    )
                    accum(pr, w, [
                        (wsb["wr"], lambda s0, sw, o=off: X9[:, x0 + o + s0 : x0 + o + s0 + sw]),
                        (wsb["ur"], lambda s0, sw, o=off: ML[:, c0 + o + s0 : c0 + o + s0 + sw]),
                    ])
                    r = ck.tile([128, CH], bf16, tag="r", name=f"lr{c}_{k}")
                    nc.scalar.activation(r[:, :w], pr[:, :w], AF.Sigmoid,
                                         bias=bsb["br"])
                    eng.tensor_mul(RM10[:, c0 + off : c0 + off + w], r[:, :w],
                                   ML[:, c0 + off : c0 + off + w])

            def zh_part(l, c, lch, S, M, Mn, RMn, X, zfirst=False):
                """Bulk chunk.  h~ first by default (its RMn inputs are ready
                at level start); z first when the child level finished just
                before (rm lands after m, so tanh's input is the late one)."""
                n = LN[l]
                c0 = c * lch
                w = min(lch, n - c0)
                seng = nc.gpsimd if S_ENG == "pool" else nc.vector

                def emit_h():
                    ph = php.tile([128, CH], f32, tag="ph", name=f"ph{l}_{c}")
                    accum(ph, w, [
                        (wsb["wh1"], lambda s0, sw: X[:, c0 + s0 : c0 + s0 + sw]),
                        (wsb["wh2"], lambda s0, sw: RMn[:, c0 + s0 : c0 + s0 + sw]),
                        (wsb["wh2"], lambda s0, sw: RMn[:, n + c0 + s0 : n + c0 + s0 + sw]),
                    ])
                    ht = ck.tile([128, CH], bf16, tag="h", name=f"ht{l}_{c}")
                    nc.scalar.activation(ht[:, :w], ph[:, :w], AF.Tanh, bias=bsb["bh"])
                    return ht

                def emit_z():
                    pz = pzp.tile([128, CH], f32, tag="pz", name=f"pz{l}_{c}")
                    accum(pz, w, [
                        (wsb["wz1"], lambda s0, sw: X[:, c0 + s0 : c0 + s0 + sw]),
                        (wsb["wz2"], lambda s0, sw: S[:, c0 + s0 : c0 + s0 + sw]),
                    ])
                    z = ck.tile([128, CH], bf16, tag="z", name=f"z{l}_{c}")
                    nc.scalar.activation(z[:, :w], pz[:, :w], AF.Sigmoid,
                                         bias=bsb["bz"])
                    return z

                if zfirst:
                    z = emit_z()
                    ht = emit_h()
                else:
                    ht = emit_h()
                    z = emit_z()
                # m = s + z*(h~ - s)
                u = ck.tile([128, CH], bf16, tag="u", name=f"u{l}_{c}")
                nc.vector.tensor_sub(u[:, :w], ht[:, :w], S[:, c0 : c0 + w])
                v = ck.tile([128, CH], bf16, tag="v", name=f"v{l}_{c}")
                nc.vector.tensor_mul(v[:, :w], z[:, :w], u[:, :w])
                nc.vector.tensor_add(M[:, c0 : c0 + w], S[:, c0 : c0 + w], v[:, :w])

            def r_part(l, c, lch, S, M, RM, Xp, pi=None):
                """r = sigmoid(Wr@x_parent + Ur@m + br); rm = r*m.  pi picks
                the psum pool (alternate across consecutive emissions)."""
                if pi is None:
                    pi = c
                n = LN[l]
                hp = LN[l - 1]
                c0 = c * lch
                w = min(lch, n - c0)
                pr = (php if pi % 2 == 0 else pzp).tile(
                    [128, CH], f32, tag="ph" if pi % 2 == 0 else "pz",
                    name=f"pr{l}_{c}"
                )
                def xp_rhs(s0, sw):
                    p0 = (c0 + s0) % hp
                    return Xp[:, p0 : p0 + sw]
                accum(pr, w, [
                    (wsb["wr"], xp_rhs),
                    (wsb["ur"], lambda s0, sw: M[:, c0 + s0 : c0 + s0 + sw]),
                ])
                r = ck.tile([128, CH], bf16, tag="r", name=f"r{l}_{c}")
                nc.scalar.activation(r[:, :w], pr[:, :w], AF.Sigmoid, bias=bsb["br"])
                rmeng = nc.gpsimd if RM_BIG_ENG == "pool" else nc.vector
                rmeng.tensor_mul(RM[:, c0 : c0 + w], r[:, :w], M[:, c0 : c0 + w])

            def small_chunk(l, S, M, RM, Mn, RMn, X, Xp):
                """Single-chunk latency form, z first: the m of the child
                level (s, z inputs) lands two hops before its rm (tanh
                input), so σ_z rides in the rm->tanh latency.  σ_r chain:
                tanh → t1 → Ur@t1 → σ_r → rm."""
                n = w = LN[l]
                hp = LN[l - 1]
                # tiny widths: Pool is idle here and has no access bubble
                te = nc.gpsimd if w <= 128 else nc.vector
                seng = nc.gpsimd if (S_ENG == "pool" or w <= 128) else nc.vector
                seng.tensor_add(S[:, :w], Mn[:, :w], Mn[:, n : n + w])
                pz = pzp.tile([128, CH], f32, tag="pz", name=f"pz{l}")
                accum(pz, w, [
                    (wsb["wz1"], lambda s0, sw: X[:, s0 : s0 + sw]),
                    (wsb["wz2"], lambda s0, sw: S[:, s0 : s0 + sw]),
                ])
                z = ck.tile([128, CH], bf16, tag="z", name=f"z{l}")
                nc.scalar.activation(z[:, :w], pz[:, :w], AF.Sigmoid, bias=bsb["bz"])
                ph = php.tile([128, CH], f32, tag="ph", name=f"ph{l}")
                accum(ph, w, [
                    (wsb["wh1"], lambda s0, sw: X[:, s0 : s0 + sw]),
                    (wsb["wh2"], lambda s0, sw: RMn[:, s0 : s0 + sw]),
                    (wsb["wh2"], lambda s0, sw: RMn[:, n + s0 : n + s0 + sw]),
                ])
                ht = ck.tile([128, CH], bf16, tag="h", name=f"ht{l}")
                nc.scalar.activation(ht[:, :w], ph[:, :w], AF.Tanh, bias=bsb["bh"])
                pr = None
                if l >= 2:
                    pr = (php if l % 2 == 0 else pzp).tile(
                        [128, CH], f32, tag="ph" if l % 2 == 0 else "pz",
                        name=f"pr{l}"
                    )
                    if w <= hp:
                        nc.tensor.matmul(pr[:, :w], wsb["wr"], Xp[:, :w],
                                         start=True, stop=False)
                    else:
                        nc.tensor.matmul(pr[:, :hp], wsb["wr"], Xp[:, :hp],
                                         start=True, stop=False)
                        nc.tensor.matmul(pr[:, hp:w], wsb["wr"], Xp[:, :hp],
                                         start=True, stop=False)
                    nc.tensor.matmul(pr[:, :w], wsb["ur"], S[:, :w],
                                     start=False, stop=False)
                t2 = ck.tile([128, CH], bf16, tag="u", name=f"t2{l}")
                te.tensor_mul(t2[:, :w], z[:, :w], S[:, :w])
                mp = ck.tile([128, CH], bf16, tag="v", name=f"mp{l}")
                te.tensor_sub(mp[:, :w], S[:, :w], t2[:, :w])
                if pr is not None:
                    nc.tensor.matmul(pr[:, :w], wsb["nur"], t2[:, :w],
                                     start=False, stop=False)
                t1 = ck.tile([128, CH], bf16, tag="t1", name=f"t1{l}")
                te.tensor_mul(t1[:, :w], z[:, :w], ht[:, :w])
                te.tensor_add(M[:, :w], mp[:, :w], t1[:, :w])
                if pr is not None:
                    nc.tensor.matmul(pr[:, :w], wsb["ur"], t1[:, :w],
                                     start=False, stop=True)
                    r = ck.tile([128, CH], bf16, tag="r", name=f"r{l}")
                    nc.scalar.activation(r[:, :w], pr[:, :w], AF.Sigmoid,
                                         bias=bsb["br"])
                    rmeng = nc.gpsimd if (RM_SMALL_ENG == "pool" or w <= 128) else nc.vector
                    rmeng.tensor_mul(RM[:, :w], r[:, :w], M[:, :w])

            # level-9 tiles: z/h parts run interleaved with the leaf chunks
            # (leaf gathers bound Pool early; keep ACT fed with level-9 work)
            M9 = mpool.tile([128, LN[9]], bf16, tag="m1", name="M9")
            RM9 = mpool.tile([128, LN[9]], bf16, tag="rm1", name="RM9")
            S9 = mpool.tile([128, LN[9]], bf16, tag="s1", name="S9")
            seng9 = nc.gpsimd if S_ENG == "pool" else nc.vector
            # r parts deferred past the X8 gathers; chunks {0,2} right after
            # zh9c1 so RM9{0,2} (level-8's first gate) lands early in the
            # dense ACT window
            R9_AT = {5: (0, 2), 7: (1, 3)}
            for i, c in enumerate(LEAF_ORDER):
                leaf_chunk(c, i, split=(i == 0))
                if i in ZH9_AT:
                    c9 = ZH9_AT[i] * CH
                    seng9.tensor_add(
                        S9[:, c9 : c9 + CH], ML[:, c9 : c9 + CH],
                        ML[:, LN[9] + c9 : LN[9] + c9 + CH]
                    )
                    zh_part(9, ZH9_AT[i], CH, S9, M9, ML, RM10, X9)
                if i in R9_AT:
                    for k, c9 in enumerate(R9_AT[i]):
                        r_part(9, c9, CH, S9, M9, RM9, xview(8), pi=k)

            # ---- levels 8..1 ----
            Mn, RMn = M9, RM9
            M1 = None
            for l in range(8, 0, -1):
                n = LN[l]
                X = xview(l)
                Xp = xview(l - 1)
                M = mpool.tile([128, n], bf16, tag=f"m{l % 2}", name=f"M{l}")
                RM = None
                if l >= 2:
                    RM = mpool.tile([128, n], bf16, tag=f"rm{l % 2}", name=f"RM{l}")
                S = mpool.tile([128, n], bf16, tag=f"s{l % 2}", name=f"S{l}")
                lch = min(512, n)
                nch = (n + lch - 1) // lch
                if l <= SMALL_MAX and nch == 1:
                    small_chunk(l, S, M, RM, Mn, RMn, X, Xp)
                else:
                    seng = nc.gpsimd if S_ENG == "pool" else nc.vector
                    for c in range(nch):
                        c0 = c * lch
                        w = min(lch, n - c0)
                        # s = m_left + m_right; inputs land early, so emit
                        # all chunks up front to keep the DVE queue unblocked
                        seng.tensor_add(
                            S[:, c0 : c0 + w], Mn[:, c0 : c0 + w],
                            Mn[:, n + c0 : n + c0 + w]
                        )
                    porder = _pair_order(nch)
                    for g0 in range(0, nch, 2):
                        grp = porder[g0 : g0 + 2]
                        for c in grp:
                            zh_part(l, c, lch, S, M, Mn, RMn, X,
                                    zfirst=(l <= 7))
                        if l >= 2:
                            for k, c in enumerate(grp):
                                r_part(l, c, lch, S, M, RM, Xp, pi=k)
                Mn, RMn = M, RM
                if l == 1:
                    M1 = M

            # ---- root readout: relu(Wg1@x_root + Wg2@(m_l + m_r) + bg) ----
            # relu+bias folded into one DVE tensor_scalar: max(pg + bg, 0)
            S0 = mpool.tile([128, T], bf16, tag="s0", name="S0")
            nc.gpsimd.tensor_add(S0[:], M1[:, :T], M1[:, T : 2 * T])
            pg = pzp.tile([128, CH], f32, tag="pz", name="pg")
            nc.tensor.matmul(pg[:, :T], wsb["wg1"], xview(0), start=True, stop=False)
            nc.tensor.matmul(pg[:, :T], wsb["wg2"], S0[:], start=False, stop=True)
            outt = ck.tile([128, T], f32, tag="o", name="outt")
            nc.vector.tensor_scalar(
                outt[:], pg[:, :T], bsb["bg"], 0.0,
                op0=mybir.AluOpType.add, op1=mybir.AluOpType.max,
            )
            nc.sync.dma_start(out=out_d[:, :], in_=outt[:])

    nc.finalize()
    return nc


def get_nc(reps=1):
    key = ("nc", reps)
    if key not in _NC_CACHE:
        _NC_CACHE[key] = _build_nc(reps)
    return _NC_CACHE[key]


def _wrap_idx(ids):
    """int16 index stream -> [16, n/16] wrapped layout."""
    return ids.astype(np.int16).reshape(-1, 16).T


def make_core_inputs(wid, emb, weights):
    import ml_dtypes

    bf16 = ml_dtypes.bfloat16
    e = np.asarray(emb, dtype=np.float32)
    Wz, Wh = np.asarray(weights["Wz_w"], np.float32), np.asarray(weights["Wh_w"], np.float32)
    Wr, Ur = np.asarray(weights["Wr_w"], np.float32), np.asarray(weights["Ur_w"], np.float32)
    Wg = np.asarray(weights["Wg_w"], np.float32)
    bz, bh = np.asarray(weights["Wz_b"], np.float32), np.asarray(weights["Wh_b"], np.float32)
    br, bg = np.asarray(weights["Ur_b"], np.float32), np.asarray(weights["Wg_b"], np.float32)

    # vocab tables (input-independent weight transforms)
    zl = 1.0 / (1.0 + np.exp(-(e @ Wz[:H] + bz)))
    hl = np.tanh(e @ Wh[:H] + bh)
    ML32 = (zl * hl).astype(np.float32)
    mltab = np.ascontiguousarray(ML32.astype(bf16))
    emb16 = np.ascontiguousarray(e.astype(bf16))

    wmats = [Wr, Ur, Wz[:H], Wz[H:], Wh[:H], Wh[H:], Wg[:H], Wg[H:], -Ur]
    wpack16 = np.concatenate(wmats, axis=1).astype(bf16)
    wbias = np.stack([bz, bh, br, bg], axis=1).astype(np.float32)

    base = {
        "emb16": emb16,
        "mltab": mltab,
        "wpack16": np.ascontiguousarray(wpack16),
        "wbias": np.ascontiguousarray(wbias),
    }

    wid = np.asarray(wid).reshape(B, NPT)
    in_maps = []
    for cid in range(NCORES):
        widc = wid[cid * T : (cid + 1) * T]  # [T, NPT]
        # per-level wids in parity (pattern-major) order
        def lvl_ids(l):
            pat = np.asarray(_PAT[l])
            return widc[np.arange(T)[None, :], pat[:, None]].ravel()  # [P*T]

        ids_by_blk = {"leaf": lvl_ids(10), "x9": lvl_ids(9), "x8": lvl_ids(8),
                      "x7": lvl_ids(7), "x6": lvl_ids(6)}
        xs = np.concatenate([lvl_ids(l) for l in _SMALL_LEVELS])
        ids_by_blk["xs"] = np.concatenate(
            [xs, np.zeros(XS_COLS - len(xs), np.int64)]
        )
        # stream = gather units in issue order
        ids = np.concatenate(
            [ids_by_blk[blk][b0 : b0 + cnt] for blk, b0, cnt in GSCHED]
        )
        assert len(ids) == GTOT
        gi = _wrap_idx(ids)  # [16, GCOLS]
        in_maps.append({**base, "gidx": np.ascontiguousarray(np.tile(gi, (8, 1)))})
    return in_maps


def kernel(**inputs):
    from concourse.bass_utils import run_bass_kernel_spmd

    nc = get_nc()
    in_maps = make_core_inputs(inputs["wid"], inputs["emb"], inputs)
    res = run_bass_kernel_spmd(nc, in_maps, core_ids=list(range(NCORES)))
    out = np.concatenate(
        [np.asarray(res.results[c]["out"]).T for c in range(NCORES)], axis=0
    )
    return np.ascontiguousarray(out.astype(np.float32))

